# revision 1
# baseline (speedup 1.0000x reference)
"""Trainium2 Bass kernel for nn_CapsuleLayer (conv per input capsule + 3-iter
dynamic routing with local 5x5 softmax pooling + squash).

Strategy: data-parallel over batch N=8 across 8 NeuronCores (one batch element
per core). Per core everything lives in SBUF:
  - conv as 5 PSUM-accumulated matmuls per y-chunk over a host-prebuilt
    dx-shifted im2col buffer (K=80 contraction, lhsT = rearranged weights)
  - u_hat[i]: [128=(o,d), 4096=(y,x)] bf16 tiles
  - routing iter 1 in closed form (b=0 -> r = 1/(8*cnt)), iters 2-3 full
  - partition-axis reductions (max/sum over o, d-group sums, broadcasts) are
    done on the TensorEngine with small pattern matmuls / transposes
  - spatial 5x5 pools as separable shifted-AP max/add trees on padded planes
Validated against the fp32 jax reference via a bit-accurate numpy model:
rel_l2 ~ 0.6% with this bf16/fp32 mix.
"""
import sys

sys.path.insert(0, "/opt/trn_rl_repo")

import numpy as np
import ml_dtypes

BF = ml_dtypes.bfloat16

NUM_IN = 8
IN_DIM = 16
KS = 5
PAD = 2
NUM_OUT = 8
OUT_DIM = 16
ROUTING = 3
N_BATCH = 8
H = 64
W_SP = 64
HP = H + 2 * PAD  # 68
SITES = H * W_SP  # 4096
OD = NUM_OUT * OUT_DIM  # 128
NCORES = 8
K80 = KS * IN_DIM  # 80

_CACHE = {}


def _patterns():
    """Host-side constant pattern matrices (all bf16)."""
    pat_acc = np.eye(128, dtype=np.float32)  # identity for PE accumulation
    # rexp[i]: lhsT [64,(o*8+i')],[128,(o*16+d)] = (i'==i): expands r (o,i)-rows to (o,d)
    pat_rexp = np.zeros((64, NUM_IN, 128), np.float32)
    for i in range(NUM_IN):
        for o in range(NUM_OUT):
            pat_rexp[o * 8 + i, i, o * 16:(o + 1) * 16] = 1.0
    # agg[i]: lhsT [128=(o,d)],[64=(o*8+i')] = (i'==i): d-group sum scattered to (o,i) row
    pat_agg = np.zeros((128, NUM_IN, 64), np.float32)
    for i in range(NUM_IN):
        for o in range(NUM_OUT):
            pat_agg[o * 16:(o + 1) * 16, i, o * 8 + i] = 1.0
    # dsum: [128=(o,d), 8=o]: d-group sum -> [o]
    pat_dsum = np.zeros((128, 8), np.float32)
    for o in range(NUM_OUT):
        pat_dsum[o * 16:(o + 1) * 16, o] = 1.0
    # osum_rep: [64=(o',i'), 64=(o,i)] = (i'==i): sum over o', replicated over o
    pat_osr = np.zeros((64, 64), np.float32)
    for i in range(NUM_IN):
        for o in range(NUM_OUT):
            pat_osr[o * 8 + i, np.arange(NUM_OUT) * 8 + i] = 0.0
    for o2 in range(NUM_OUT):
        for i in range(NUM_IN):
            for op_ in range(NUM_OUT):
                pat_osr[op_ * 8 + i, o2 * 8 + i] = 1.0
    # e2: [8=i', 64=(o,i)] = (i'==i): expand per-i row to (o,i) rows
    pat_e2 = np.zeros((8, 64), np.float32)
    for i in range(NUM_IN):
        for o in range(NUM_OUT):
            pat_e2[i, o * 8 + i] = 1.0
    # sexp: [8=o, 128=(o,d)]: expand squash scale over d
    pat_sexp = np.zeros((8, 128), np.float32)
    for o in range(NUM_OUT):
        pat_sexp[o, o * 16:(o + 1) * 16] = 1.0
    # cnt map for iter-1 closed form: r = 1/(NUM_OUT*cnt)
    yy, xx = np.meshgrid(np.arange(H), np.arange(W_SP), indexing="ij")
    cy = np.minimum(yy, PAD) + np.minimum(H - 1 - yy, PAD) + 1
    cx = np.minimum(xx, PAD) + np.minimum(W_SP - 1 - xx, PAD) + 1
    cntinv = (1.0 / (NUM_OUT * cy * cx)).astype(np.float32).reshape(SITES)
    cntinv = np.broadcast_to(cntinv, (128, SITES)).copy()
    ident64 = np.eye(64, dtype=np.float32)
    return {
        "pat_acc": pat_acc.astype(BF),
        "pat_rexp": pat_rexp.astype(BF),
        "pat_agg": pat_agg.astype(BF),
        "pat_dsum": pat_dsum.astype(BF),
        "pat_osr": pat_osr.astype(BF),
        "pat_sexp": pat_sexp.astype(BF),
        "pat_e2": pat_e2.astype(BF),
        "cntinv": cntinv.astype(BF),
        "ident64": ident64.astype(np.float32),
    }


def _host_prep(u, W):
    """Build per-core conv inputs with 25 taps packed into 4 matmuls:
    B8 [(dy0-1,dx0-3)x16c = 128 rows] covers tap groups {dy,dy+1}x{0..3} at
    row offsets 0 / 2*HP / (rows 0:64 only) 4*HP; B5 [(dy0-4)x16c = 80 rows]
    covers the dx=4 column. Returns b8 [N,I,128,HP*HP], b5 [N,I,80,HP*HP],
    w12 [I,128,2,OD], w3 [I,64,OD], w4 [I,80,OD], all bf16."""
    ub = u.astype(BF).astype(np.float32)
    up = np.zeros((N_BATCH, NUM_IN, IN_DIM, HP, HP), np.float32)
    up[:, :, :, PAD:PAD + H, PAD:PAD + W_SP] = ub
    b8 = np.zeros((N_BATCH, NUM_IN, 128, HP, HP), np.float32)
    for dy in range(2):
        for dx in range(4):
            t = dy * 4 + dx
            b8[:, :, t * 16:(t + 1) * 16, :HP - dy, :HP - dx] = \
                up[:, :, :, dy:, dx:]
    b5 = np.zeros((N_BATCH, NUM_IN, K80, HP, HP), np.float32)
    for dy in range(KS):
        b5[:, :, dy * 16:(dy + 1) * 16, :HP - dy, :HP - 4] = \
            up[:, :, :, dy:, 4:]
    b8 = b8.reshape(N_BATCH, NUM_IN, 128, HP * HP).astype(BF)
    b5 = b5.reshape(N_BATCH, NUM_IN, K80, HP * HP).astype(BF)
    wb = W.astype(BF).astype(np.float32)
    w12 = np.zeros((NUM_IN, 128, 2, OD), np.float32)
    for g in range(2):
        for dy in range(2):
            for dx in range(4):
                t = dy * 4 + dx
                w12[:, t * 16:(t + 1) * 16, g, :] = \
                    np.transpose(wb[:, :, :, 2 * g + dy, dx], (0, 2, 1))
    w3 = np.zeros((NUM_IN, 64, OD), np.float32)
    for dx in range(4):
        w3[:, dx * 16:(dx + 1) * 16, :] = \
            np.transpose(wb[:, :, :, 4, dx], (0, 2, 1))
    w4 = np.zeros((NUM_IN, K80, OD), np.float32)
    for dy in range(KS):
        w4[:, dy * 16:(dy + 1) * 16, :] = \
            np.transpose(wb[:, :, :, dy, 4], (0, 2, 1))
    return (b8, b5, w12.astype(BF), w3.astype(BF), w4.astype(BF))


def _build_program():
    import concourse.bass as bass
    import concourse.bacc as bacc
    import concourse.mybir as mybir
    from concourse import tile

    fp32 = mybir.dt.float32
    bf16 = mybir.dt.bfloat16
    AF = mybir.ActivationFunctionType
    ALU = mybir.AluOpType
    PSUM = bass.MemorySpace.PSUM

    nc = bacc.Bacc("TRN2", target_bir_lowering=False, debug=False,
                   num_devices=NCORES)

    b8_d = nc.declare_dram_parameter("b8", [NUM_IN, 128, HP * HP], bf16, False)
    b5_d = nc.declare_dram_parameter("b5", [NUM_IN, K80, HP * HP], bf16, False)
    w12_d = nc.declare_dram_parameter("w12", [NUM_IN, 128, 2, OD], bf16, False)
    w3_d = nc.declare_dram_parameter("w3", [NUM_IN, 64, OD], bf16, False)
    w4_d = nc.declare_dram_parameter("w4", [NUM_IN, K80, OD], bf16, False)
    pacc_d = nc.declare_dram_parameter("pat_acc", [128, 128], bf16, False)
    prexp_d = nc.declare_dram_parameter("pat_rexp", [64, NUM_IN, 128], bf16, False)
    pagg_d = nc.declare_dram_parameter("pat_agg", [128, NUM_IN, 64], bf16, False)
    pdsum_d = nc.declare_dram_parameter("pat_dsum", [128, 8], bf16, False)
    posr_d = nc.declare_dram_parameter("pat_osr", [64, 64], bf16, False)
    psexp_d = nc.declare_dram_parameter("pat_sexp", [8, 128], bf16, False)
    pe2_d = nc.declare_dram_parameter("pat_e2", [8, 64], bf16, False)
    cntinv_d = nc.declare_dram_parameter("cntinv", [128, SITES], bf16, False)
    id64_d = nc.declare_dram_parameter("ident64", [64, 64], fp32, False)
    vout_d = nc.declare_dram_parameter("vout", [OD, SITES], fp32, True)

    with tile.TileContext(nc) as tc:
        with tc.tile_pool(name="const", bufs=1) as cst, \
             tc.tile_pool(name="state", bufs=1) as st:

            patacc = cst.tile([128, 128], bf16)
            nc.sync.dma_start(patacc[:], pacc_d.ap())
            patrexp = cst.tile([64, NUM_IN, 128], bf16)
            nc.sync.dma_start(patrexp[:], prexp_d.ap())
            patagg = cst.tile([128, NUM_IN, 64], bf16)
            nc.sync.dma_start(patagg[:], pagg_d.ap())
            patdsum = cst.tile([128, 8], bf16)
            nc.sync.dma_start(patdsum[:], pdsum_d.ap())
            patosr = cst.tile([64, 64], bf16)
            nc.sync.dma_start(patosr[:], posr_d.ap())
            patsexp = cst.tile([8, 128], bf16)
            nc.sync.dma_start(patsexp[:], psexp_d.ap())
            pate2 = cst.tile([8, 64], bf16)
            nc.sync.dma_start(pate2[:], pe2_d.ap())
            cntinv = cst.tile([128, SITES], bf16)
            nc.sync.dma_start(cntinv[:], cntinv_d.ap())
            id64 = cst.tile([64, 64], fp32)
            nc.sync.dma_start(id64[:], id64_d.ap())
            eps_bias = cst.tile([8, 1], fp32)
            nc.gpsimd.memset(eps_bias[:], 1e-9)

            uhat = [st.tile([128, SITES], bf16, tag=f"uhat{i}", name=f"uhat{i}")
                    for i in range(NUM_IN)]
            bten = st.tile([64, SITES], fp32, tag="bten")
            bmax_pad = st.tile([8, HP, HP], bf16, tag="bmaxpad")
            cs_pad = st.tile([64, HP, HP], bf16, tag="cspad")
            bmax8 = st.tile([8, SITES], bf16, tag="bmax8")
            wxs = st.tile([64, HP, W_SP], bf16, tag="wxs")
            cten = st.tile([64, SITES], bf16, tag="cten")
            rten = st.tile([64, SITES], bf16, tag="rten")

            # initial -inf border for the max pool pad (re-done per iter after
            # it is reused as sum-tree scratch); cs_pad is memset in-loop
            nc.gpsimd.memset(bmax_pad[:], -1e30)

            # ---------------- conv: u_hat[i] ----------------
            with tc.tile_pool(name="convio", bufs=2) as cio, \
                 tc.tile_pool(name="cpsum", bufs=1, space=PSUM) as cps:
                for i in range(NUM_IN):
                    b8t = cio.tile([128, HP, HP], bf16, tag="b8t")
                    nc.sync.dma_start(b8t[:], b8_d.ap()[i])
                    b5t = cio.tile([K80, HP, HP], bf16, tag="b5t")
                    nc.sync.dma_start(b5t[:], b5_d.ap()[i])
                    w12t = cio.tile([128, 2, OD], bf16, tag="w12t")
                    nc.sync.dma_start(w12t[:], w12_d.ap()[i])
                    w3t = cio.tile([64, OD], bf16, tag="w3t")
                    nc.sync.dma_start(w3t[:], w3_d.ap()[i])
                    w4t = cio.tile([K80, OD], bf16, tag="w4t")
                    nc.sync.dma_start(w4t[:], w4_d.ap()[i])
                    for yc in range(8):
                        y0 = yc * 8
                        ps = cps.tile([128, 8, 64], fp32, tag="acc", bufs=2)
                        nc.tensor.matmul(
                            ps[:], w12t[:, 0, :],
                            b8t[:, y0:y0 + 8, 0:W_SP],
                            start=True, stop=False)
                        nc.tensor.matmul(
                            ps[:], w12t[:, 1, :],
                            b8t[:, y0 + 2:y0 + 10, 0:W_SP],
                            start=False, stop=False)
                        nc.tensor.matmul(
                            ps[:], w3t[:],
                            b8t[0:64, y0 + 4:y0 + 12, 0:W_SP],
                            start=False, stop=False)
                        nc.tensor.matmul(
                            ps[:], w4t[:],
                            b5t[:, y0:y0 + 8, 0:W_SP],
                            start=False, stop=True)
                        dst = uhat[i][:, yc * 512:(yc + 1) * 512]
                        if (i + yc) % 2:
                            nc.vector.tensor_copy(dst, ps[:])
                        else:
                            nc.scalar.copy(dst, ps[:])

            with tc.tile_pool(name="work", bufs=2) as wk, \
                 tc.tile_pool(name="rpsum", bufs=2, space=PSUM) as rps, \
                 tc.tile_pool(name="xpsum", bufs=3, space=PSUM) as xps, \
                 tc.tile_pool(name="bpsum", bufs=2, space=PSUM) as bps:

                def chunk_tail(it, yc, pp):
                    """squash(p-chunk) [+ agree + b update, or final DMA]."""
                    last = (it == ROUTING - 1)
                    sl = slice(yc * 512, (yc + 1) * 512)
                    p_c = wk.tile([128, 512], bf16, tag="p_c")
                    nc.scalar.copy(p_c[:], pp[:])
                    sq_c = wk.tile([128, 512], bf16, tag="sq_c")
                    nc.scalar.activation(sq_c[:], p_c[:], AF.Square)
                    nsqp = xps.tile([8, 8, 64], fp32, tag="aux")
                    nc.tensor.matmul(nsqp[:], patdsum[:], sq_c[:],
                                     start=True, stop=True)
                    nsq_c = wk.tile([8, 512], fp32, tag="nsq_c")
                    nc.scalar.copy(nsq_c[:], nsqp[:])
                    dent_c = wk.tile([8, 512], fp32, tag="dent_c")
                    nc.scalar.activation(dent_c[:], nsq_c[:], AF.Sqrt,
                                         bias=eps_bias[:])
                    nc.vector.scalar_tensor_tensor(
                        dent_c[:], nsq_c[:], 1.0, dent_c[:],
                        op0=ALU.add, op1=ALU.mult)
                    rden_c = wk.tile([8, 512], fp32, tag="rden_c")
                    nc.vector.reciprocal_approx_fast(rden_c[:], dent_c[:])
                    scale_c = wk.tile([8, 512], bf16, tag="scale_c")
                    nc.vector.tensor_mul(scale_c[:], nsq_c[:], rden_c[:])
                    scp = xps.tile([128, 8, 64], fp32, tag="aux")
                    nc.tensor.matmul(scp[:], patsexp[:], scale_c[:],
                                     start=True, stop=True)
                    if last:
                        vchunk = wk.tile([128, 512], fp32, tag="vchunk")
                        nc.vector.tensor_mul(vchunk[:], p_c[:], scp[:])
                        nc.sync.dma_start(vout_d.ap()[:, sl], vchunk[:])
                        return
                    v_c = wk.tile([128, 512], bf16, tag="v_c")
                    nc.vector.tensor_mul(v_c[:], p_c[:], scp[:])
                    ag = bps.tile([64, 8, 64], fp32, tag="bacc")
                    for i in range(NUM_IN):
                        prod = wk.tile([128, 512], bf16, tag="prod")
                        nc.vector.tensor_mul(prod[:], uhat[i][:, sl], v_c[:])
                        nc.tensor.matmul(ag[:], patagg[:, i, :], prod[:],
                                         start=(i == 0), stop=(i == NUM_IN - 1))
                    if it == 0:
                        nc.scalar.copy(bten[:, sl], ag[:])
                    else:
                        nc.vector.tensor_tensor(bten[:, sl], bten[:, sl],
                                                ag[:], op=ALU.add)

                # ---------------- iter 1 (b = 0 closed form) ----------------
                for yc in range(8):
                    sl = slice(yc * 512, (yc + 1) * 512)
                    pp = rps.tile([128, 8, 64], fp32, tag="pacc")
                    for i in range(NUM_IN):
                        nc.tensor.matmul(pp[:], patacc[:], uhat[i][:, sl],
                                         start=(i == 0), stop=(i == NUM_IN - 1))
                    ppb = wk.tile([128, 512], bf16, tag="ppb")
                    nc.vector.tensor_mul(ppb[:], pp[:], cntinv[:, sl])
                    pp2 = rps.tile([128, 8, 64], fp32, tag="pacc")
                    nc.tensor.matmul(pp2[:], patacc[:], ppb[:],
                                     start=True, stop=True)
                    chunk_tail(0, yc, pp2)

                # ---------------- routing iters 2..ROUTING ----------------
                for it in range(1, ROUTING):
                    # bmax: o-max via PE transpose/reduce/replicate + 5x5 max
                    for ci in range(32):
                        cl = slice(ci * 128, (ci + 1) * 128)
                        tp = xps.tile([128, 64], fp32, tag="aux")
                        nc.tensor.transpose(tp[:], bten[:, cl], id64[:])
                        red = wk.tile([128, 8], bf16, tag="red")
                        nc.vector.tensor_reduce(
                            red[:],
                            tp[:].rearrange("p (o i) -> p i o", o=8),
                            axis=mybir.AxisListType.X, op=ALU.max)
                        mm = xps.tile([8, 128], bf16, tag="auxb", bufs=1)
                        nc.tensor.matmul(mm[:], red[:], patacc[:],
                                         start=True, stop=True,
                                         is_transpose=True)
                        y0 = 2 * ci
                        nc.scalar.copy(
                            bmax_pad[:, PAD + y0:PAD + y0 + 2, PAD:PAD + W_SP],
                            mm[:].rearrange("p (y x) -> p y x", y=2))
                    T = bmax_pad
                    t1 = wk.tile([8, HP, 66], bf16, tag="treeA", bufs=1)
                    nc.vector.tensor_max(t1[:], T[:, :, 0:66], T[:, :, 1:67])
                    t2 = wk.tile([8, HP, 64], bf16, tag="treeB", bufs=1)
                    nc.vector.tensor_max(t2[:], t1[:, :, 0:64], t1[:, :, 2:66])
                    tx = cs_pad[0:8, :, 0:64]  # scratch: cs_pad dead here
                    nc.vector.tensor_max(tx, t2[:], T[:, :, 4:68])
                    u1 = wk.tile([8, 66, 64], bf16, tag="treeA", bufs=1)
                    nc.vector.tensor_max(u1[:], tx[0:8, 0:66, :], tx[0:8, 1:67, :])
                    u2 = wk.tile([8, 64, 64], bf16, tag="treeB", bufs=1)
                    nc.vector.tensor_max(u2[:], u1[:, 0:64, :], u1[:, 2:66, :])
                    nc.vector.tensor_max(
                        bmax8[:].rearrange("p (y x) -> p y x", y=H),
                        u2[:], tx[0:8, 4:68, :])

                    # c = exp(b - expand(bmax)), chunked; expansion read
                    # straight from PSUM
                    for yc in range(8):
                        sl = slice(yc * 512, (yc + 1) * 512)
                        bxp = xps.tile([64, 8, 64], fp32, tag="aux")
                        nc.tensor.matmul(bxp[:], pate2[:], bmax8[:, sl],
                                         start=True, stop=True)
                        subt = wk.tile([64, 512], bf16, tag="subt")
                        nc.vector.tensor_sub(
                            subt[:], bten[:, sl],
                            bxp[:].rearrange("p a b -> p (a b)"))
                        nc.scalar.activation(cten[:, sl], subt[:], AF.Exp)

                    # restore cs_pad borders (tx scratch clobbered them)
                    nc.gpsimd.memset(cs_pad[:], 0.0)

                    # sum_c: o-sum (replicated over o) into cs_pad + 5x5 box sum
                    for yc in range(8):
                        sl = slice(yc * 512, (yc + 1) * 512)
                        osp = xps.tile([64, 8, 64], fp32, tag="aux")
                        nc.tensor.matmul(osp[:], patosr[:], cten[:, sl],
                                         start=True, stop=True)
                        nc.scalar.copy(
                            cs_pad[:, PAD + yc * 8:PAD + yc * 8 + 8,
                                   PAD:PAD + W_SP],
                            osp[:])
                    Tc = cs_pad
                    w1 = wk.tile([64, HP, 66], bf16, tag="treeA", bufs=1)
                    nc.vector.tensor_add(w1[:], Tc[:, :, 0:66], Tc[:, :, 1:67])
                    w2 = wk.tile([64, HP, 64], bf16, tag="treeB", bufs=1)
                    nc.vector.tensor_add(w2[:], w1[:, :, 0:64], w1[:, :, 2:66])
                    wx = wxs
                    nc.vector.tensor_add(wx[:], w2[:], Tc[:, :, 4:68])
                    y1 = wk.tile([64, 66, 64], bf16, tag="treeA", bufs=1)
                    nc.vector.tensor_add(y1[:], wx[:, 0:66, :], wx[:, 1:67, :])
                    y2 = wk.tile([64, 64, 64], bf16, tag="treeB", bufs=1)
                    nc.vector.tensor_add(y2[:], y1[:, 0:64, :], y1[:, 2:66, :])
                    # final box op + recip + r, chunked over y-blocks
                    for yc in range(8):
                        sl = slice(yc * 512, (yc + 1) * 512)
                        yr = slice(yc * 8, yc * 8 + 8)
                        sumc_c = wk.tile([64, 8, 64], fp32, tag="sumc_c")
                        nc.vector.tensor_add(sumc_c[:], y2[:, yr, :],
                                             wx[:, yc * 8 + 4:yc * 8 + 12, :])
                        recip_c = wk.tile([64, 8, 64], fp32, tag="recip_c")
                        nc.vector.reciprocal_approx_fast(recip_c[:], sumc_c[:])
                        recipb_c = wk.tile([64, 512], bf16, tag="recipb_c")
                        nc.vector.tensor_copy(recipb_c[:],
                                              recip_c[:].rearrange("p a b -> p (a b)"))
                        nc.vector.tensor_mul(rten[:, sl], cten[:, sl],
                                             recipb_c[:])

                    # p = sum_i expand_d(r_i) * u_hat_i, then squash/agree
                    for yc in range(8):
                        sl = slice(yc * 512, (yc + 1) * 512)
                        pp = rps.tile([128, 8, 64], fp32, tag="pacc")
                        for i in range(NUM_IN):
                            rp = xps.tile([128, 8, 64], fp32, tag="aux")
                            nc.tensor.matmul(rp[:], patrexp[:, i, :],
                                             rten[:, sl], start=True, stop=True)
                            reb = wk.tile([128, 512], bf16, tag="reb", bufs=3)
                            nc.scalar.copy(reb[:], rp[:])
                            q = wk.tile([128, 512], bf16, tag="q", bufs=3)
                            nc.vector.tensor_mul(q[:], uhat[i][:, sl], reb[:])
                            nc.tensor.matmul(pp[:], patacc[:], q[:],
                                             start=(i == 0),
                                             stop=(i == NUM_IN - 1))
                        chunk_tail(it, yc, pp)

    nc.compile()
    return nc


def _get_program():
    if "nc" not in _CACHE:
        _CACHE["nc"] = _build_program()
    return _CACHE["nc"]


def kernel(u, W):
    u = np.asarray(u, np.float32)
    W = np.asarray(W, np.float32)
    nc = _get_program()
    pats = _patterns()
    b8, b5, w12, w3, w4 = _host_prep(u, W)
    in_maps = []
    for n in range(NCORES):
        m = {"b8": np.ascontiguousarray(b8[n]),
             "b5": np.ascontiguousarray(b5[n]),
             "w12": w12, "w3": w3, "w4": w4}
        m.update(pats)
        in_maps.append(m)

    from concourse.bass_utils import run_bass_kernel_spmd
    res = run_bass_kernel_spmd(nc, in_maps, core_ids=list(range(NCORES)))
    out = np.stack([res.results[n]["vout"] for n in range(NCORES)])
    return out.reshape(N_BATCH, NUM_OUT, OUT_DIM, H, W_SP).astype(np.float32)

